# revision 7
# baseline (speedup 1.0000x reference)
"""Trainium2 Bass kernel for ControlFlowGraphEncoder (GNN + per-graph BiLSTM).

Sharding: data-parallel over the 128 graphs -> 16 graphs (8192 nodes) per core.
All weights replicated. Edges are graph-local so no cross-core traffic.

Per-core layout choices:
  - x kept transposed: xT [256 feat, 8192 nodes] as 2 SBUF tiles [128, 8192]
  - segment ops done as dense per-graph matmuls against host-built
    normalized adjacency A_hat (A_hat[d,s] = count(d,s)/max(deg(d),1)):
        agg @ Wl.T = A_hat @ (x @ Wl.T)  (associativity)
  - SAGE layer: zT[m] = sum_k WrT[k,m-chunk] path + y-path, gelu in ACT
  - BiLSTM: 512 fully unrolled steps, batch = 16 graphs x 2 dirs in one
    PSUM tile [48, 512] (rows 0:16 fwd, 32:48 bwd, 16:32 dead), gate
    column order (i, f, o, g). PX = x @ Wih.T + biases precomputed to DRAM
    and streamed in 8-step windows. h kept transposed via PE transpose.
"""

import numpy as np

import concourse.bass as bass
import concourse.tile as tile
import concourse.mybir as mybir
from concourse import bacc, bass_utils

F32 = mybir.dt.float32
AF = mybir.ActivationFunctionType
OP = mybir.AluOpType

NC = 8          # cores
G = 128         # graphs total
GP = 16         # graphs per core
L = 512         # nodes per graph
NP = GP * L     # nodes per core = 8192
FIN = 128
H = 256
HH = 128
WIN = 8         # lstm window steps
NW = L // WIN   # 64 windows

_cached = {}


def _build_nc():
    nc = bacc.Bacc("TRN2", target_bir_lowering=False, debug=False,
                   enable_asserts=False)

    def inp(name, shape):
        return nc.dram_tensor(name, shape, F32, kind="ExternalInput").ap()

    def outp(name, shape):
        return nc.dram_tensor(name, shape, F32, kind="ExternalOutput").ap()

    nfT = inp("nfT", [128, NP])
    eT = inp("eT", [64, NP])
    at_in = inp("at", [GP, 4, 128, L])
    wpT = inp("wpT", [128, H])
    bpT = inp("bpT", [128, 2])
    wlT = inp("wlT", [3, 2, 128, H])
    wrT = inp("wrT", [3, 2, 2, 128, 128])
    blT = inp("blT", [128, 6])
    wihT = inp("wihT", [2, 2, 128, 4 * HH])
    whhT = inp("whhT", [2, 128, 4 * HH])
    inj_f = inp("inj_f", [17, 32])
    inj_b = inp("inj_b", [17, 16])
    id48 = inp("id48", [48, 48])
    biasf = inp("biasf", [1, WIN * 4 * HH])
    biasb = inp("biasb", [1, WIN * 4 * HH])
    wheadT = inp("wheadT", [2, 128, 7])
    bhead = inp("bhead", [128, 7])

    o_xc = outp("o_xc", [2, 128, NP])
    o_xl = outp("o_xl", [2, 128, NP])
    o_ge = outp("o_ge", [2, 128, GP])
    o_pr = outp("o_pr", [1, GP * 7])

    pxf = nc.dram_tensor("pxf", [GP, L, 4 * HH], F32, kind="Internal").ap()
    pxb = nc.dram_tensor("pxb", [GP, L, 4 * HH], F32, kind="Internal").ap()

    with tile.TileContext(nc) as tc:
        _build_body(tc, nc, locals())
    nc.compile()
    return nc


def _build_body(tc, nc, t):
    nfT, eT, at_in = t["nfT"], t["eT"], t["at_in"]
    wpT, bpT, wlT, wrT, blT = t["wpT"], t["bpT"], t["wlT"], t["wrT"], t["blT"]
    wihT, whhT = t["wihT"], t["whhT"]
    inj_f, inj_b, id48 = t["inj_f"], t["inj_b"], t["id48"]
    biasf, biasb = t["biasf"], t["biasb"]
    wheadT, bhead = t["wheadT"], t["bhead"]
    o_xc, o_xl, o_ge, o_pr = t["o_xc"], t["o_xl"], t["o_ge"], t["o_pr"]
    pxf, pxb = t["pxf"], t["pxb"]

    import contextlib
    ctx = contextlib.ExitStack()
    with ctx:
        big = ctx.enter_context(tc.tile_pool(name="big", bufs=1))
        wts = ctx.enter_context(tc.tile_pool(name="wts", bufs=1))

        # persistent x buffer (transposed [feat, node]); xa is scoped below
        xb = [big.tile([128, NP], F32, tag=f"xb{m}", name=f"xb{m}") for m in range(2)]
        xap = tc.alloc_tile_pool(name="xap", bufs=1)
        xa = [xap.tile([128, NP], F32, tag=f"xa{m}", name=f"xa{m}") for m in range(2)]

        # weights to SBUF
        wpT_sb = wts.tile([128, H], F32)
        nc.sync.dma_start(out=wpT_sb, in_=wpT)
        bpT_sb = wts.tile([128, 2], F32)
        nc.sync.dma_start(out=bpT_sb, in_=bpT)
        wlT_sb = wts.tile([128, 3, 2, H], F32)
        nc.sync.dma_start(out=wlT_sb, in_=wlT.rearrange("l k p f -> p l k f"))
        wrT_sb = wts.tile([128, 3, 2, 2, 128], F32)
        nc.sync.dma_start(out=wrT_sb, in_=wrT.rearrange("l k m p f -> p l k m f"))
        blT_sb = wts.tile([128, 6], F32)
        nc.sync.dma_start(out=blT_sb, in_=blT)
        wihT_sb = wts.tile([128, 2, 2, 4 * HH], F32)
        nc.sync.dma_start(out=wihT_sb, in_=wihT.rearrange("d k p f -> p d k f"))
        whhT_sb = wts.tile([128, 2, 4 * HH], F32)
        nc.sync.dma_start(out=whhT_sb, in_=whhT.rearrange("d p f -> p d f"))
        injf_sb = wts.tile([17, 32], F32)
        nc.sync.dma_start(out=injf_sb, in_=inj_f)
        injb_sb = wts.tile([17, 16], F32)
        nc.sync.dma_start(out=injb_sb, in_=inj_b)
        id48_sb = wts.tile([48, 48], F32)
        nc.sync.dma_start(out=id48_sb, in_=id48)
        wheadT_sb = wts.tile([128, 2, 7], F32)
        nc.sync.dma_start(out=wheadT_sb, in_=wheadT.rearrange("k p f -> p k f"))
        bhead_sb = wts.tile([128, 7], F32)
        nc.sync.dma_start(out=bhead_sb, in_=bhead)
        z16 = wts.tile([128, 16], F32)
        nc.vector.memset(z16, 0.0)

        # ---------------- projection + embedding ----------------
        with tc.tile_pool(name="projin", bufs=3) as projin, \
             tc.tile_pool(name="ps_z", bufs=4, space="PSUM") as pz0:
            for s in range(16):
                sl = slice(s * 512, (s + 1) * 512)
                nt = projin.tile([128, 512], F32, tag="nt")
                nc.sync.dma_start(out=nt, in_=nfT[:, sl])
                for m in range(2):
                    pp = pz0.tile([128, 512], F32, tag="pp")
                    nc.tensor.matmul(pp, wpT_sb[:, m * 128:(m + 1) * 128],
                                     nt, start=True, stop=True)
                    nc.vector.tensor_scalar_add(xa[m][:, sl], pp,
                                                bpT_sb[:, m:m + 1])
            for s4 in range(4):
                sl = slice(s4 * 2048, (s4 + 1) * 2048)
                et = projin.tile([64, 2048], F32, tag="et")
                nc.sync.dma_start(out=et, in_=eT[:, sl])
                nc.vector.tensor_add(xa[0][0:64, sl], xa[0][0:64, sl], et)

        # ---------------- 3 GNN layers ----------------
        with tc.tile_pool(name="atp", bufs=3) as atp, \
             tc.tile_pool(name="yp", bufs=2) as yp, \
             tc.tile_pool(name="ps_y", bufs=4, space="PSUM") as py, \
             tc.tile_pool(name="ps_z2", bufs=4, space="PSUM") as pz:
            xcur, xnext = xa, xb
            for lyr in range(3):
                for g in range(GP):
                    gs = slice(g * L, (g + 1) * L)
                    at_t = atp.tile([128, 4, L], F32, tag="at")
                    nc.sync.dma_start(out=at_t,
                                      in_=at_in[g].rearrange("c p d -> p c d"))
                    y_sb = yp.tile([128, 4, H], F32, tag="y")
                    for c in range(4):
                        cs = slice(g * L + c * 128, g * L + (c + 1) * 128)
                        py_ = py.tile([128, H], F32, tag="py")
                        nc.tensor.matmul(py_, xcur[0][:, cs],
                                         wlT_sb[:, lyr, 0, :], start=True, stop=False)
                        nc.tensor.matmul(py_, xcur[1][:, cs],
                                         wlT_sb[:, lyr, 1, :], start=False, stop=True)
                        nc.scalar.copy(y_sb[:, c, :], py_)
                    for m in range(2):
                        pz_ = pz.tile([128, L], F32, tag="pzl")
                        nc.tensor.matmul(pz_, wrT_sb[:, lyr, 0, m, :],
                                         xcur[0][:, gs], start=True, stop=False)
                        nc.tensor.matmul(pz_, wrT_sb[:, lyr, 1, m, :],
                                         xcur[1][:, gs], start=False, stop=False)
                        for c in range(4):
                            nc.tensor.matmul(pz_,
                                             y_sb[:, c, m * 128:(m + 1) * 128],
                                             at_t[:, c, :], start=False, stop=(c == 3))
                        nc.scalar.activation(xnext[m][:, gs], pz_, AF.Gelu,
                                             bias=blT_sb[:, lyr * 2 + m:
                                                         lyr * 2 + m + 1])
                xcur, xnext = xnext, xcur
        xfin = xcur  # after 3 layers this is xb
        assert xfin is xb
        xap.release()

        # ---------------- PX = x @ WihT + bias (to DRAM) ----------------
        with tc.tile_pool(name="pxsb", bufs=4) as pxsb, \
             tc.tile_pool(name="ps_px", bufs=4, space="PSUM") as ppxp:
            for d, pxd in ((0, pxf), (1, pxb)):
                for ch in range(64):
                    g, t0 = ch // 4, (ch % 4) * 128
                    cs = slice(ch * 128, (ch + 1) * 128)
                    ppx = ppxp.tile([128, 4 * HH], F32, tag="ppx")
                    nc.tensor.matmul(ppx, xfin[0][:, cs], wihT_sb[:, d, 0, :],
                                     start=True, stop=False)
                    nc.tensor.matmul(ppx, xfin[1][:, cs], wihT_sb[:, d, 1, :],
                                     start=False, stop=True)
                    spx = pxsb.tile([128, 4 * HH], F32, tag="spx")
                    nc.any.tensor_copy(spx, ppx)
                    nc.sync.dma_start(out=pxd[g, t0:t0 + 128, :], in_=spx)

        # ---------------- BiLSTM, 512 unrolled steps ----------------
        with tc.tile_pool(name="wtp", bufs=2) as wtp, \
             tc.tile_pool(name="histp", bufs=2) as histp, \
             tc.tile_pool(name="lstw", bufs=3) as lstw, \
             tc.tile_pool(name="ps_g", bufs=2, space="PSUM") as pg, \
             tc.tile_pool(name="ps_h", bufs=2, space="PSUM") as ph:
            c_sb = big.tile([48, HH], F32, tag="c_st")
            nc.vector.memset(c_sb, 0.0)
            hist_prev = None  # (histf, histb) of previous window
            for w in range(NW):
                tlo_b = L - WIN - WIN * w  # base t of bwd window
                wtf = wtp.tile([17, WIN, 4 * HH], F32, tag="wtf")
                wtb = wtp.tile([17, WIN, 4 * HH], F32, tag="wtb")
                nc.sync.dma_start(out=wtf[0:16],
                                  in_=pxf[:, WIN * w:WIN * (w + 1), :])
                nc.sync.dma_start(
                    out=wtf[16:17],
                    in_=biasf.rearrange("p (b d) -> p b d", d=4 * HH))
                nc.sync.dma_start(out=wtb[0:16],
                                  in_=pxb[:, tlo_b:tlo_b + WIN, :])
                nc.sync.dma_start(
                    out=wtb[16:17],
                    in_=biasb.rearrange("p (b d) -> p b d", d=4 * HH))
                histf = histp.tile([128, 16, WIN], F32, tag="hf")
                histb = histp.tile([128, 16, WIN], F32, tag="hb")
                for b in range(WIN):
                    tau = w * WIN + b
                    if tau == 0:
                        hpf = hpb = z16
                    elif b == 0:
                        hpf = hist_prev[0][:, :, WIN - 1]
                        hpb = hist_prev[1][:, :, 0]
                    else:
                        hpf = histf[:, :, b - 1]
                        hpb = histb[:, :, WIN - b]
                    gates = pg.tile([48, 4 * HH], F32, tag="gates")
                    nc.tensor.matmul(gates[0:32, :], injf_sb, wtf[:, b, :],
                                     start=True, stop=False, skip_group_check=True)
                    nc.tensor.matmul(gates[0:16, :], hpf, whhT_sb[:, 0, :],
                                     start=False, stop=True,
                                     skip_group_check=True)
                    nc.tensor.matmul(gates[32:48, :], injb_sb,
                                     wtb[:, WIN - 1 - b, :], start=True,
                                     stop=False, tile_position=(0, 32),
                                     skip_group_check=True)
                    nc.tensor.matmul(gates[32:48, :], hpb, whhT_sb[:, 1, :],
                                     start=False, stop=True,
                                     tile_position=(0, 32),
                                     skip_group_check=True)
                    sg = lstw.tile([48, 3 * HH], F32, tag="sg")
                    nc.scalar.activation(sg, gates[:, 0:3 * HH], AF.Sigmoid)
                    tg = lstw.tile([48, HH], F32, tag="tg")
                    nc.scalar.activation(tg, gates[:, 3 * HH:4 * HH], AF.Tanh)
                    t2 = lstw.tile([48, HH], F32, tag="t2")
                    nc.vector.tensor_mul(t2, sg[:, 0:HH], tg)
                    t1 = lstw.tile([48, HH], F32, tag="t1")
                    nc.vector.tensor_mul(t1, sg[:, HH:2 * HH], c_sb)
                    nc.vector.tensor_add(c_sb, t1, t2)
                    tc_ = lstw.tile([48, HH], F32, tag="tc")
                    nc.scalar.activation(tc_, c_sb, AF.Tanh)
                    h_sb = lstw.tile([48, HH], F32, tag="h")
                    nc.vector.tensor_mul(h_sb, sg[:, 2 * HH:3 * HH], tc_)
                    hT = ph.tile([128, 48], F32, tag="hT")
                    nc.tensor.transpose(hT, h_sb, id48_sb)
                    nc.vector.tensor_copy(histf[:, :, b], hT[:, 0:16])
                    nc.vector.tensor_copy(histb[:, :, WIN - 1 - b],
                                          hT[:, 32:48])
                # flush window hists to x_lstm in DRAM
                dstf = o_xl[0].rearrange("p (g t) -> p g t", t=L)
                nc.sync.dma_start(
                    out=dstf[:, :, WIN * w:WIN * (w + 1)], in_=histf)
                dstb = o_xl[1].rearrange("p (g t) -> p g t", t=L)
                nc.sync.dma_start(
                    out=dstb[:, :, tlo_b:tlo_b + WIN], in_=histb)
                hist_prev = (histf, histb)

        # ---------------- combine, outputs, heads ----------------
        tc.strict_bb_all_engine_barrier()
        with tc.tile_pool(name="lbp", bufs=3) as lbp:
            for m in range(2):
                for s in range(4):
                    sl = slice(s * 2048, (s + 1) * 2048)
                    lt = lbp.tile([128, 2048], F32, tag="lt")
                    nc.sync.dma_start(out=lt, in_=o_xl[m][:, sl])
                    nc.vector.tensor_add(xfin[m][:, sl], xfin[m][:, sl], lt)
        xcomb = xfin
        for m in range(2):
            nc.sync.dma_start(out=o_xc[m], in_=xcomb[m])

        with tc.tile_pool(name="hdp", bufs=1) as hdp, \
             tc.tile_pool(name="hds", bufs=4) as hds, \
             tc.tile_pool(name="ps_l", bufs=4, space="PSUM") as pl:
            # graph embedding (mean over nodes)
            for m in range(2):
                ger = hds.tile([128, GP], F32, tag="ge")
                nc.vector.tensor_reduce(
                    ger, xcomb[m].rearrange("p (g t) -> p g t", t=L),
                    axis=mybir.AxisListType.X, op=OP.add)
                nc.vector.tensor_scalar_mul(ger, ger, 1.0 / L)
                nc.sync.dma_start(out=o_ge[m], in_=ger)
            # per-node softmax heads -> per-graph means
            e_all = hdp.tile([128, 64, 7], F32)
            for ch in range(64):
                cs = slice(ch * 128, (ch + 1) * 128)
                pl_ = pl.tile([128, 7], F32, tag="pl")
                nc.tensor.matmul(pl_, xcomb[0][:, cs], wheadT_sb[:, 0, :],
                                 start=True, stop=False)
                nc.tensor.matmul(pl_, xcomb[1][:, cs], wheadT_sb[:, 1, :],
                                 start=False, stop=True)
                le = hds.tile([128, 7], F32, tag="le")
                nc.vector.tensor_add(le, pl_, bhead_sb)
                nc.scalar.activation(e_all[:, ch, :], le, AF.Exp)
            dbr = hdp.tile([128, 64], F32)
            nc.vector.tensor_reduce(dbr, e_all[:, :, 0:3],
                                    axis=mybir.AxisListType.X, op=OP.add)
            dlp = hdp.tile([128, 64], F32)
            nc.vector.tensor_reduce(dlp, e_all[:, :, 3:7],
                                    axis=mybir.AxisListType.X, op=OP.add)
            rbr = hdp.tile([128, 64], F32)
            nc.vector.reciprocal(rbr, dbr)
            rlp = hdp.tile([128, 64], F32)
            nc.vector.reciprocal(rlp, dlp)
            pr = hdp.tile([128, 64, 7], F32)
            for j in range(7):
                nc.vector.tensor_mul(pr[:, :, j], e_all[:, :, j],
                                     rbr if j < 3 else rlp)
            prsum = hdp.tile([1, 64 * 7], F32)
            nc.gpsimd.tensor_reduce(prsum, pr.rearrange("p c j -> p (c j)"),
                                    axis=mybir.AxisListType.C, op=OP.add)
            prv = prsum.rearrange("p (g cc j) -> p g cc j", cc=4, j=7)
            acc = hdp.tile([1, GP, 7], F32)
            nc.vector.tensor_add(acc, prv[:, :, 0, :], prv[:, :, 1, :])
            nc.vector.tensor_add(acc, acc, prv[:, :, 2, :])
            nc.vector.tensor_add(acc, acc, prv[:, :, 3, :])
            nc.vector.tensor_scalar_mul(acc, acc, 1.0 / L)
            nc.sync.dma_start(out=o_pr,
                              in_=acc.rearrange("p g j -> p (g j)"))


# ======================= host side =======================

PERM = np.concatenate([np.arange(0, 128), np.arange(128, 256),
                       np.arange(384, 512), np.arange(256, 384)])


def _host_prep(inputs):
    """Build the per-core input maps (all numpy float32)."""
    nf = np.asarray(inputs["node_features"], np.float32)
    ei = np.asarray(inputs["edge_index"])
    bt = np.asarray(inputs["block_types"])
    emb = np.asarray(inputs["emb_table"], np.float32)

    src, dst = ei[0], ei[1]
    deg = np.bincount(dst, minlength=G * L).astype(np.float32)
    cnt = np.maximum(deg, 1.0)
    # dense per-graph A_hat.T:  AT[g, s, d] = count(d,s)/cnt(d)
    a = np.zeros((G, L, L), np.float32)
    gidx = (dst // L).astype(np.int64)
    np.add.at(a, (gidx, dst % L, src % L), 1.0)
    a /= cnt.reshape(G, L, 1)
    at_full = np.ascontiguousarray(np.transpose(a, (0, 2, 1)))  # [G, s, d]

    wp = np.asarray(inputs["W_proj"], np.float32)       # [H, F]
    bp = np.asarray(inputs["b_proj"], np.float32)       # [H]
    wl = np.asarray(inputs["sage_Wl"], np.float32)      # [3, H, H]
    bl = np.asarray(inputs["sage_bl"], np.float32)      # [3, H]
    wr = np.asarray(inputs["sage_Wr"], np.float32)

    def gate_w(w):  # [4HH, X] -> permuted transpose [X, 4HH]
        return np.ascontiguousarray(w[PERM].T.astype(np.float32))

    wih = [gate_w(np.asarray(inputs["Wih_f"], np.float32)),
           gate_w(np.asarray(inputs["Wih_b"], np.float32))]   # [256, 512]
    whh = [gate_w(np.asarray(inputs["Whh_f"], np.float32)),
           gate_w(np.asarray(inputs["Whh_b"], np.float32))]   # [128, 512]
    bcomb = [
        (np.asarray(inputs["bih_f"], np.float32)
         + np.asarray(inputs["bhh_f"], np.float32))[PERM],
        (np.asarray(inputs["bih_b"], np.float32)
         + np.asarray(inputs["bhh_b"], np.float32))[PERM],
    ]
    wbr = np.asarray(inputs["W_branch"], np.float32)    # [3, H]
    bbr = np.asarray(inputs["b_branch"], np.float32)
    wlp = np.asarray(inputs["W_loop"], np.float32)      # [4, H]
    blp = np.asarray(inputs["b_loop"], np.float32)
    whead = np.concatenate([wbr, wlp], 0)               # [7, H]
    bhead = np.concatenate([bbr, blp], 0)               # [7]

    shared = {
        "wpT": np.ascontiguousarray(wp.T),                       # [128, 256]
        "bpT": np.ascontiguousarray(bp.reshape(2, 128).T),       # [128, 2]
        "wlT": np.ascontiguousarray(
            np.stack([w.T for w in wl]).reshape(3, 2, 128, H)),
        "wrT": np.ascontiguousarray(
            np.stack([w.T for w in wr]).reshape(3, 2, 128, 2, 128)
            .transpose(0, 1, 3, 2, 4)),                          # [3,2,2,128,128]
        "blT": np.ascontiguousarray(bl.reshape(3, 2, 128)
                                    .transpose(2, 0, 1).reshape(128, 6)),
        "wihT": np.ascontiguousarray(
            np.stack(wih).reshape(2, 2, 128, 4 * HH)),
        "whhT": np.ascontiguousarray(np.stack(whh)),             # [2,128,512]
        "inj_f": np.ascontiguousarray(np.vstack(
            [np.hstack([np.eye(16, dtype=np.float32),
                        np.zeros((16, 16), np.float32)]),
             np.hstack([np.ones((1, 16), np.float32),
                        np.zeros((1, 16), np.float32)])])),      # [17, 32]
        "inj_b": np.ascontiguousarray(np.vstack(
            [np.eye(16, dtype=np.float32),
             np.ones((1, 16), np.float32)])),                    # [17, 16]
        "id48": np.eye(48, dtype=np.float32),
        "biasf": np.ascontiguousarray(
            np.tile(bcomb[0], WIN).reshape(1, -1)),
        "biasb": np.ascontiguousarray(
            np.tile(bcomb[1], WIN).reshape(1, -1)),
        "wheadT": np.ascontiguousarray(whead.T.reshape(2, 128, 7)),
        "bhead": np.ascontiguousarray(
            np.tile(bhead.reshape(1, 7), (128, 1))),
    }
    # blT check: bl[l] is [256] -> chunks [2,128]; partition p of chunk m is
    # bl[l][m*128+p] -> reshape(3,2,128).transpose(2,0,1) gives [128,(l,m)]. ok

    in_maps = []
    for c in range(NC):
        ns = slice(c * NP, (c + 1) * NP)
        m = dict(shared)
        m["nfT"] = np.ascontiguousarray(nf[ns].T)                # [128, 8192]
        m["eT"] = np.ascontiguousarray(emb[bt[ns]].T)            # [64, 8192]
        # at[g, c, p, d] = at_full[g][128c+p, d]  (s-axis split c-major)
        m["at"] = np.ascontiguousarray(
            at_full[c * GP:(c + 1) * GP].reshape(GP, 4, 128, L))
        in_maps.append(m)
    return in_maps


def kernel(**inputs):
    if "nc" not in _cached:
        _cached["nc"] = _build_nc()
    nc = _cached["nc"]
    in_maps = _host_prep(inputs)
    res = bass_utils.run_bass_kernel_spmd(nc, in_maps,
                                          core_ids=list(range(NC)))
    outs = res.results
    N = G * L
    x_comb = np.empty((N, H), np.float32)
    x_lstm = np.empty((N, H), np.float32)
    ge = np.empty((G, H), np.float32)
    br = np.empty((G, 3), np.float32)
    lp = np.empty((G, 4), np.float32)
    for c in range(NC):
        o = outs[c]
        ns = slice(c * NP, (c + 1) * NP)
        gs = slice(c * GP, (c + 1) * GP)
        x_comb[ns] = o["o_xc"].reshape(H, NP).T
        x_lstm[ns] = o["o_xl"].reshape(H, NP).T
        ge[gs] = o["o_ge"].reshape(H, GP).T
        pr = o["o_pr"].reshape(GP, 7)
        br[gs] = pr[:, 0:3]
        lp[gs] = pr[:, 3:7]
    return x_comb, ge, br, lp, x_lstm
